# revision 5
# baseline (speedup 1.0000x reference)
"""Trainium2 Bass kernel for the Arcface loss forward.

Math (from the reference):
  xn = x / ||x||_F                     (global frobenius norm over the whole tensor)
  for each unordered pair (i<j) of the S axis:
      scores[b,(i,j)] = 5 * xn[b,i] @ xn[b,j].T          # [W, W]
      tgt[b,(i,j),w]  = first v with target[b,j,v] == target[b,i,w], else 0

Outputs: scores [B*P*W, W] f32, tgt [B*P*W] int32  (P = S*(S-1)/2 = 120)

Strategy: data-parallel over B across 8 cores (4 batches per core).
 - scale = sqrt(5)/||x|| is folded into both GEMM operands (each side gets
   sqrt(scale... i.e. each side is multiplied by sqrt(5)/||x|| so the product
   carries 5/||x||^2).
 - x[b,s] tiles are PE-transposed ([W,h] -> [h,W]) so the contraction dim h
   sits on partitions; compute dtype bf16 (fp32 accumulate in PSUM).
 - tgt via exact small matmuls: first-index table per (b,s) row built with
   vector compares + max-reduce; gathered through a one-hot matmul.
"""

import math

import numpy as np

import concourse.bass as bass
import concourse.mybir as mybir
import concourse.tile as tile
from concourse import bacc
from concourse.bass_utils import run_bass_kernel_spmd
from concourse.masks import make_identity

# problem shape (hardcoded per harness contract)
B, S, W, H = 32, 16, 128, 256
P = S * (S - 1) // 2  # 120
N_CORES = 8
BL = B // N_CORES  # 4 batches per core
ROWS = BL * P * W  # 61440 output rows per core
NSH = 2  # h halves of 128

DEVICE_NORM = False  # v1: host computes the global-norm scale

FP = mybir.dt.float32
BF = mybir.dt.bfloat16
I32 = mybir.dt.int32
ALU = mybir.AluOpType
AF = mybir.ActivationFunctionType


def _pbase(i):
    # first pair index with first element i in row-major triu order
    return sum(S - 1 - k for k in range(i))


def build_nc():
    nc = bacc.Bacc("TRN2", target_bir_lowering=False, debug=False)

    x_d = nc.dram_tensor("x", [BL, S, W, H], FP, kind="ExternalInput")
    t_d = nc.dram_tensor("target", [BL, S, W], I32, kind="ExternalInput")
    sc_d = nc.dram_tensor("scale", [1, 1], FP, kind="ExternalInput")
    scores_d = nc.dram_tensor("scores", [ROWS, W], FP, kind="ExternalOutput")
    tgt_d = nc.dram_tensor("tgt", [ROWS], I32, kind="ExternalOutput")

    if DEVICE_NORM:
        cc_in = nc.dram_tensor("cc_in", [1, 1], FP)
        cc_out = nc.dram_tensor("cc_out", [1, 1], FP, addr_space="Shared")

    with tile.TileContext(nc) as tc:
        with (
            tc.tile_pool(name="consts", bufs=1) as consts,
            tc.tile_pool(name="xraw", bufs=BL) as xraw_pool,
            tc.tile_pool(name="xt", bufs=BL * NSH) as xt_pool,
            tc.tile_pool(name="outs", bufs=3) as out_pool,
            tc.tile_pool(name="small", bufs=2) as small,
            tc.tile_pool(name="intp", bufs=4) as intp,
            tc.tile_pool(name="ps_mm", bufs=4, space="PSUM") as ps_mm,
            tc.tile_pool(name="ps_tp", bufs=2, space="PSUM") as ps_tp,
            tc.tile_pool(name="ps_int", bufs=2, space="PSUM") as ps_int,
        ):
            # ---------------- constants ----------------
            ident = consts.tile([128, 128], FP)
            make_identity(nc, ident[:])

            # iota_c[c, w] = c  (16 partitions)
            iota_c = consts.tile([16, W], FP)
            nc.gpsimd.iota(
                iota_c[:], pattern=[[0, W]], base=0, channel_multiplier=1,
                allow_small_or_imprecise_dtypes=True,
            )
            # wv[r, w] = W - w  (weight that makes max-reduce pick first match)
            wv = consts.tile([BL * S, W], FP)
            nc.gpsimd.iota(
                wv[:], pattern=[[-1, W]], base=W, channel_multiplier=0,
                allow_small_or_imprecise_dtypes=True,
            )
            # E64[k, (r, c)] = 1 if k == r else 0   (row-selector weights)
            e64_i = consts.tile([BL * S, BL * S, 16], I32)
            nc.gpsimd.iota(
                e64_i[:], pattern=[[1, BL * S], [0, 16]], base=0,
                channel_multiplier=-1,
            )
            e64 = consts.tile([BL * S, BL * S * 16], BF)
            nc.vector.tensor_scalar(
                e64[:].rearrange("k (r c) -> k r c", c=16), e64_i[:], 0.0, None,
                op0=ALU.is_equal,
            )
            ones128 = consts.tile([128, 1], FP)
            nc.gpsimd.memset(ones128[:], 1.0)
            ones_row = consts.tile([1, 128], FP)
            nc.gpsimd.memset(ones_row[:], 1.0)

            # ---------------- input loads ----------------
            xraw = []
            for b in range(BL):
                xr = xraw_pool.tile([W, S, H], FP, tag="xraw")
                nc.sync.dma_start(xr[:], x_d[b].rearrange("s w h -> w s h"))
                xraw.append(xr)

            tmat_i = small.tile([BL * S, W], I32)
            nc.sync.dma_start(tmat_i[:], t_d.ap().rearrange("b s w -> (b s) w"))
            tmat_bf = small.tile([BL * S, W], BF)
            nc.vector.tensor_copy(tmat_bf[:], tmat_i[:])
            tmat_f = small.tile([BL * S, W], FP)
            nc.vector.tensor_copy(tmat_f[:], tmat_i[:])

            # ---------------- norm scale ----------------
            scale_sb = small.tile([1, 1], FP)
            if not DEVICE_NORM:
                nc.sync.dma_start(scale_sb[:], sc_d[:, :])
            else:
                parts = []
                for b in range(BL):
                    scratch = small.tile([W, S, H], FP, tag="sq_scratch")
                    part = small.tile([128, 1], FP, tag="sq_part")
                    nc.scalar.activation(
                        scratch[:], xraw[b][:], AF.Square, accum_out=part[:]
                    )
                    parts.append(part)
                tot = small.tile([128, 1], FP)
                nc.vector.tensor_tensor(tot[:], parts[0][:], parts[1][:], op=ALU.add)
                nc.vector.tensor_tensor(tot[:], tot[:], parts[2][:], op=ALU.add)
                nc.vector.tensor_tensor(tot[:], tot[:], parts[3][:], op=ALU.add)
                ps_ss = ps_int.tile([1, 1], FP, tag="ps_int")
                nc.tensor.matmul(ps_ss[:], tot[:], ones128[:], start=True, stop=True)
                ss_sb = small.tile([1, 1], FP)
                nc.vector.tensor_copy(ss_sb[:], ps_ss[:])
                nc.sync.dma_start(cc_in[:, :], ss_sb[:])
                nc.gpsimd.collective_compute(
                    "AllReduce",
                    ALU.add,
                    ins=[cc_in[:, :]],
                    outs=[cc_out[:, :]],
                    replica_groups=[list(range(N_CORES))],
                )
                g_sb = small.tile([1, 1], FP)
                nc.sync.dma_start(g_sb[:], cc_out[:, :])
                r_sb = small.tile([1, 1], FP)
                nc.vector.reciprocal(r_sb[:], g_sb[:])  # 1/sumsq
                r5_sb = small.tile([1, 1], FP)
                nc.vector.tensor_scalar(
                    r5_sb[:], r_sb[:], 5.0, None, op0=ALU.mult
                )  # 5/sumsq
                nc.scalar.activation(scale_sb[:], r5_sb[:], AF.Sqrt)

            # broadcast scale to all 128 partitions via ones @ scale
            ps_bc = ps_int.tile([128, 1], FP, tag="ps_int")
            nc.tensor.matmul(
                ps_bc[:], ones_row[:], scale_sb[:], start=True, stop=True,
            )
            scale128 = consts.tile([128, 1], FP)
            nc.vector.tensor_copy(scale128[:], ps_bc[:])

            # ---------------- integer target path ----------------
            # first-index table: fi[(b,s), c] = min{w : target[b,s,w]==c} else 0
            maxv = intp.tile([BL * S, 16], FP, tag="maxv")
            for c in range(16):
                eq = intp.tile([BL * S, W], FP, tag="eq")
                nc.vector.tensor_scalar(
                    eq[:], tmat_f[:], float(c), None, op0=ALU.is_equal
                )
                val = intp.tile([BL * S, W], FP, tag="val")
                nc.vector.tensor_tensor(val[:], eq[:], wv[:], op=ALU.mult)
                nc.vector.reduce_max(
                    maxv[:, c : c + 1], val[:], axis=mybir.AxisListType.X
                )
            gt0 = intp.tile([BL * S, 16], FP, tag="gt0")
            nc.vector.tensor_scalar(gt0[:], maxv[:], 0.0, None, op0=ALU.is_gt)
            wmin = intp.tile([BL * S, 16], FP, tag="wmin")
            nc.vector.tensor_scalar(
                wmin[:], maxv[:], -1.0, float(W), op0=ALU.mult, op1=ALU.add
            )
            fi = intp.tile([BL * S, 16], FP, tag="fi")
            nc.vector.tensor_tensor(fi[:], gt0[:], wmin[:], op=ALU.mult)
            ps_fi = ps_int.tile([16, BL * S], FP, tag="ps_int")
            nc.tensor.transpose(ps_fi[:], fi[:], ident[0 : BL * S, 0 : BL * S])
            fiT = intp.tile([16, BL * S], BF, tag="fiT")
            nc.vector.tensor_copy(fiT[:], ps_fi[:])

            tgt2d = tgt_d.ap().rearrange("(r v) -> r v", v=W)
            for b in range(BL):
                for i in range(S - 1):
                    row = b * S + i
                    L = S - 1 - i
                    ps_rep = ps_int.tile([16, W], FP, tag="ps_int")
                    nc.tensor.matmul(
                        ps_rep[:],
                        e64[:, row * 16 : (row + 1) * 16],
                        tmat_bf[:],
                        start=True, stop=True,
                    )
                    cmp = intp.tile([16, W], BF, tag="cmp")
                    nc.vector.tensor_tensor(
                        cmp[:], ps_rep[:], iota_c[:], op=ALU.is_equal
                    )
                    ps_tgt = ps_int.tile([16, W], FP, tag="ps_int")
                    nc.tensor.matmul(
                        ps_tgt[:],
                        fiT[:, b * S : (b + 1) * S],
                        cmp[:],
                        start=True, stop=True,
                    )
                    tgt_i = intp.tile([16, W], I32, tag="tgt_i")
                    nc.vector.tensor_copy(tgt_i[:], ps_tgt[:])
                    r0 = b * P + _pbase(i)
                    nc.sync.dma_start(
                        tgt2d[r0 : r0 + L, :], tgt_i[i + 1 : S, :]
                    )

            # ---------------- transpose + scale/cast to bf16 ----------------
            xt = []  # xt[b][half] : [128(h), S*W] bf16
            for b in range(BL):
                pair = []
                for hf in range(NSH):
                    t = xt_pool.tile([128, S * W], BF, tag="xt")
                    pair.append(t)
                xt.append(pair)
            for b in range(BL):
                for s in range(S):
                    for hf in range(NSH):
                        ps_t = ps_tp.tile([128, 128], FP, tag="ps_tp")
                        nc.tensor.transpose(
                            ps_t[:],
                            xraw[b][:, s, hf * 128 : (hf + 1) * 128],
                            ident[:],
                        )
                        nc.vector.tensor_scalar(
                            xt[b][hf][:, s * W : (s + 1) * W],
                            ps_t[:],
                            scale128[:],
                            None,
                            op0=ALU.mult,
                        )

            # ---------------- pair GEMMs ----------------
            scores2d = scores_d.ap().rearrange("(pp w) v -> w pp v", w=W)
            drain_k = 0
            for b in range(BL):
                for i in range(S - 1):
                    j0 = i + 1
                    L = S - 1 - i
                    stage = out_pool.tile([128, L * W], FP, tag="outs")
                    # chunks of up to 4 pair-tiles (<=512 psum columns)
                    bounds = list(range(0, L, 4)) + [L]
                    chunks = list(zip(bounds[:-1], bounds[1:]))
                    psums = []
                    for (c0, c1) in chunks:
                        n = (c1 - c0) * W
                        pt = ps_mm.tile([128, 512], FP, tag="ps_mm")
                        psums.append(pt)
                        nc.tensor.matmul(
                            pt[:, :n],
                            xt[b][0][:, i * W : (i + 1) * W],
                            xt[b][0][:, (j0 + c0) * W : (j0 + c1) * W],
                            start=True, stop=False,
                        )
                    for (c0, c1), pt in zip(chunks, psums):
                        n = (c1 - c0) * W
                        nc.tensor.matmul(
                            pt[:, :n],
                            xt[b][1][:, i * W : (i + 1) * W],
                            xt[b][1][:, (j0 + c0) * W : (j0 + c1) * W],
                            start=False, stop=True,
                        )
                    for (c0, c1), pt in zip(chunks, psums):
                        n = (c1 - c0) * W
                        dst = stage[:, c0 * W : c1 * W]
                        # balance PSUM drain across scalar + vector engines
                        if drain_k % 5 < 3:
                            nc.scalar.copy(dst, pt[:, :n])
                        else:
                            nc.vector.tensor_copy(dst, pt[:, :n])
                        drain_k += 1
                    r0 = b * P + _pbase(i)
                    nc.sync.dma_start(
                        scores_d.ap()[r0 * W : (r0 + L) * W, :].rearrange(
                            "(pp w) v -> w pp v", w=W
                        ),
                        stage[:].rearrange("w (pp v) -> w pp v", v=W),
                    )

    nc.compile()
    return nc


_NC = None


def _get_nc():
    global _NC
    if _NC is None:
        _NC = build_nc()
    return _NC


def kernel(x: np.ndarray, target: np.ndarray):
    x = np.ascontiguousarray(x, dtype=np.float32)
    target = np.ascontiguousarray(target, dtype=np.int32)

    scale = np.float32(math.sqrt(5.0) / math.sqrt(float(np.sum(np.square(x, dtype=np.float64)))))
    sc = np.array([[scale]], dtype=np.float32)

    in_maps = []
    for c in range(N_CORES):
        in_maps.append(
            {
                "x": np.ascontiguousarray(x[c * BL : (c + 1) * BL]),
                "target": np.ascontiguousarray(target[c * BL : (c + 1) * BL]),
                "scale": sc,
            }
        )

    nc = _get_nc()
    res = run_bass_kernel_spmd(nc, in_maps, core_ids=list(range(N_CORES)))
    scores = np.concatenate([r["scores"] for r in res.results], axis=0)
    tgt = np.concatenate([r["tgt"] for r in res.results], axis=0)
    return scores, tgt
